# revision 30
# baseline (speedup 1.0000x reference)
"""GQA attention block on 8 NeuronCores (Trainium2, Bass/Tile).

Sharding: tensor-parallel over head groups (4 ways: 8 q heads / 2 kv heads
per core) x data-parallel over batch (2 ways).  Each core computes a partial
y = attn_out_slice @ Wo_slice for its (batch, head-group); the host sums the
4 TP partials per batch element.

Per-core device program:
  A) x^T via f32r PE transposes (batched 8-per-PSUM-tile, evictions split
     between the Vector and Scalar engines, cast to bf16); q^T/k^T/v^T
     projections in bf16 (fast weight load).  All 4 x row-tiles of a chunk
     are DMA-prefetched up front; weight DMAs are ordered K,V,Q to match
     first use.
  B) attention with two-head software pipelining: heads run in pairs with
     interleaved S -> exp -> PV chains, and PV lags one ki-group behind
     S/exp so the PE never stalls on the Scalar engine.  exp runs on
     [128, 1024] PSUM tiles (2 banks, 2 key-tiles per instruction, scale
     1/sqrt(dh) folded into the activation) to amortize the ~352-cycle ACT
     instruction overhead.  Softmax denominators come from a ones-row in
     the PV weights; normalization = SBUF-staged reciprocal_approx_fast +
     gpsimd partition broadcast + DVE multiply (custom-DVE ops cannot read
     PSUM on hardware).
  C) y = out^T.T @ Wo (bf16), with block tb's 64 matmuls drip-fed 2 per
     inner iteration into block tb+1's attention loop to fill PE gaps and
     keep the PE clock warm.

Measured on trn2: ~497 us HW exec (baseline: 907 us), rel err ~5e-3.
"""

import os
import sys

import ml_dtypes
import numpy as np

for _p in ("/opt/trn_rl_repo",):
    if os.path.isdir(_p) and _p not in sys.path:
        sys.path.insert(0, _p)

from contextlib import ExitStack

import concourse.bass as bass  # noqa: F401  (AP types pulled in transitively)
import concourse.mybir as mybir
import concourse.tile as tile
from concourse import bacc
from concourse.bass_utils import run_bass_kernel_spmd
from concourse.masks import make_identity

P = 128
B, T, D = 2, 2048, 2048
HQ, HKV, DH = 32, 8, 64
GROUP = HQ // HKV            # 4
TP = 4                       # tensor-parallel ways
DP = 2                       # data-parallel ways
NCORES = TP * DP
DQ = D // TP                 # 512 q dims per core (8 heads)
DKV = HKV * DH // TP         # 128 kv dims per core (2 kv heads)
NHQ = HQ // TP               # 8 q heads per core
NKV = HKV // TP              # 2 kv heads per core
NKS = D // P                 # 16 contraction subtiles over D
CH = 512                     # T chunk width in projection phase
NCH = T // CH                # 4
TQB = 512                    # T_q block width in attention / psum bank
NTQB = T // TQB              # 4
NKI = T // P                 # 16 key tiles
NB = D // 512                # 4 output column banks
SCALE = 1.0 / 8.0            # 1/sqrt(DH), folded into Wq on the host
F32 = mybir.dt.float32
F32R = mybir.dt.float32r
BF16 = mybir.dt.bfloat16
FP8 = mybir.dt.float8e4
I32 = mybir.dt.int32
AF = mybir.ActivationFunctionType
PM = mybir.MatmulPerfMode
# Schraudolph fast-exp: bitcast(int32(EXA*x + EXB)) ~ exp(x) (max rel err ~3%)
EXA = 12102203.16 * SCALE     # 2^23/ln2, with the 1/sqrt(dh) folded in
EXB = 127.0 * 2**23 - 366390.0


def _build():
    nc = bacc.Bacc(None, target_bir_lowering=False, debug=False)

    x_ext = nc.dram_tensor("x", [T, D], F32, kind="ExternalInput")
    wq_ext = nc.dram_tensor("wq", [D, DQ], BF16, kind="ExternalInput")
    wk_ext = nc.dram_tensor("wk", [D, DKV], BF16, kind="ExternalInput")
    wv_ext = nc.dram_tensor("wv", [D, DKV], BF16, kind="ExternalInput")
    wo_ext = nc.dram_tensor("wo", [DQ, D], BF16, kind="ExternalInput")
    y_ext = nc.dram_tensor("y", [T, D], F32, kind="ExternalOutput")

    x_v = x_ext[:].rearrange("(to p) d -> p to d", p=P)      # [128,16,2048]
    wq_v = wq_ext[:].rearrange("(ko p) m -> p ko m", p=P)    # [128,16,512]
    wk_v = wk_ext[:].rearrange("(ko p) m -> p ko m", p=P)    # [128,16,128]
    wv_v = wv_ext[:].rearrange("(ko p) m -> p ko m", p=P)
    wo_v = wo_ext[:].rearrange("(ko p) n -> p ko n", p=P)    # [128,4,2048]
    y_v = y_ext[:].rearrange("(to p) n -> p to n", p=P)      # [128,16,2048]

    with tile.TileContext(nc) as tc, ExitStack() as ctx:
        const = ctx.enter_context(tc.tile_pool(name="const", bufs=1))
        big = ctx.enter_context(tc.tile_pool(name="big", bufs=3))
        wkv_p = ctx.enter_context(tc.tile_pool(name="wkv", bufs=1))
        row_p = ctx.enter_context(tc.tile_pool(name="rows", bufs=4))
        qt_p = ctx.enter_context(tc.tile_pool(name="qt", bufs=1))
        kt_p = ctx.enter_context(tc.tile_pool(name="kt", bufs=1))
        vo_p = ctx.enter_context(tc.tile_pool(name="vo", bufs=1))
        exp_p = ctx.enter_context(tc.tile_pool(name="expp", bufs=8))
        bc_p = ctx.enter_context(tc.tile_pool(name="bcp", bufs=2))
        rc_p = ctx.enter_context(tc.tile_pool(name="rcp", bufs=2))
        ot_p = ctx.enter_context(tc.tile_pool(name="otp", bufs=2))

        # PSUM: 8 banks total.
        # s_ps: 2 x [128,1024] (2 banks each) -> 4 banks; attention scores,
        #       also phase-A transpose staging.
        # pv_ps: 2 x [128,512] -> 2 banks; PV accumulators, also phase-A
        #       v-transpose staging.
        # proj_ps: 2 x [128,512] -> 2 banks; projections + Wo outputs.
        s_ps = ctx.enter_context(tc.tile_pool(name="s_ps", bufs=2, space="PSUM"))
        pv_ps = ctx.enter_context(tc.tile_pool(name="pv_ps", bufs=2, space="PSUM"))
        proj_ps = ctx.enter_context(tc.tile_pool(name="proj_ps", bufs=2, space="PSUM"))

        identity = const.tile([P, P], F32)
        make_identity(nc, identity)
        identity_r = const.tile([P, P], F32R)
        nc.vector.tensor_copy(identity_r[:], identity[:])
        id_r = identity_r[:]
        ones_col = const.tile([P, NKV, NKI], F32)
        nc.gpsimd.memset(ones_col[:], 1.0)

        wq_sb = big.tile([P, NKS, DQ], BF16, tag="big")
        wk_sb = wkv_p.tile([P, NKS, DKV], BF16, tag="wk")
        wv_sb = wkv_p.tile([P, NKS, DKV], BF16, tag="wv")

        qt_sb = qt_p.tile([P, DQ // P, T], BF16)   # q^T, [dim, t]
        kt_sb = kt_p.tile([P, T], BF16)            # k^T, [dim(2 kv), t]
        vones = vo_p.tile([P, NKV, NKI, DH + 1], BF16)  # [t%128, kv, t//128, dh|1]
        nc.vector.tensor_copy(vones[:, :, :, DH], ones_col[:])

        # preload the exp table set early so the first phase-B exp is cheap
        warm = rc_p.tile([1, 8], F32, tag="rc")
        nc.gpsimd.memset(warm[:], 0.0)
        nc.scalar.activation(warm[:], warm[:], AF.Exp)

        # ---- Phase A: x^T chunks + projections ----
        for c in range(NCH):
            xt_ch = big.tile([P, NKS, CH], BF16, tag="big")  # x^T[:, c*CH:+CH]
            # prefetch all 4 x row-tiles of this chunk up front
            xrows = []
            for r in range(CH // P):
                xrow = row_p.tile([P, D], F32R, tag="rows", name=f"xrow{r}")
                nc.sync.dma_start(
                    xrow[:], x_v[:, c * (CH // P) + r, :].bitcast(F32R))
                xrows.append(xrow)
            if c == 0:
                # weight DMAs after the first x rows, in need order (K,V,Q);
                # wq in halves so Q-proj mb0/1 can start sooner
                nc.sync.dma_start(wk_sb[:], wk_v)
                nc.sync.dma_start(wv_sb[:], wv_v)
                nc.sync.dma_start(wq_sb[:, :, 0:DQ // 2], wq_v[:, :, 0:DQ // 2])
                nc.sync.dma_start(wq_sb[:, :, DQ // 2:], wq_v[:, :, DQ // 2:])
            for r in range(CH // P):
                xrow_r = xrows[r][:]
                for half in range(2):
                    tp = s_ps.tile([P, 8, P], F32R, tag="s")
                    for k in range(8):
                        dsb = half * 8 + k
                        nc.tensor.transpose(
                            tp[:, k, :], xrow_r[:, dsb * P:(dsb + 1) * P], id_r)
                    # batched eviction of 8 transposes; alternate DVE/ACT
                    dst = xt_ch[:, half * 8:(half + 1) * 8, r * P:(r + 1) * P]
                    if half == 0:
                        nc.vector.tensor_copy(dst, tp[:].bitcast(F32))
                    else:
                        nc.scalar.activation(dst, tp[:].bitcast(F32), AF.Copy)
            # k^T chunk
            kp = proj_ps.tile([P, CH], F32, tag="proj")
            for ks in range(NKS):
                nc.tensor.matmul(kp[:], wk_sb[:, ks, :],
                                 xt_ch[:, ks, :],
                                 start=(ks == 0), stop=(ks == NKS - 1))
            nc.scalar.activation(kt_sb[:, c * CH:(c + 1) * CH], kp[:], AF.Copy)
            # v^T chunk, then PE-transpose into vones (v in natural [t, dh] layout)
            vp = proj_ps.tile([P, CH], F32, tag="proj")
            for ks in range(NKS):
                nc.tensor.matmul(vp[:], wv_sb[:, ks, :],
                                 xt_ch[:, ks, :],
                                 start=(ks == 0), stop=(ks == NKS - 1))
            vt_sb = row_p.tile([P, CH], F32, tag="vt")
            nc.vector.tensor_copy(vt_sb[:], vp[:])
            tpv = pv_ps.tile([P, 4, P], F32, tag="pv")
            for r in range(CH // P):
                nc.tensor.transpose(
                    tpv[:, r, :], vt_sb[:, r * P:(r + 1) * P], identity)
            for j in range(NKV):
                nc.vector.tensor_copy(
                    vones[:, j, c * 4:(c + 1) * 4, 0:DH],
                    tpv[:, :, j * DH:(j + 1) * DH])
            # q^T chunk (scale already folded into Wq host-side)
            for mb in range(DQ // P):
                qp = proj_ps.tile([P, CH], F32, tag="proj")
                for ks in range(NKS):
                    nc.tensor.matmul(
                        qp[:], wq_sb[:, ks, mb * P:(mb + 1) * P],
                        xt_ch[:, ks, :],
                        start=(ks == 0), stop=(ks == NKS - 1))
                nc.vector.tensor_copy(qt_sb[:, mb, c * CH:(c + 1) * CH], qp[:])

        # prefetch Wo (slot freed by wq after phase A)
        wo_sb = big.tile([P, DQ // P, D], BF16, tag="big")
        nc.sync.dma_start(wo_sb[:], wo_v)

        # ---- Phases B+C interleaved per T_q block ----
        # q heads are permuted host-side to order [0,4,1,5,2,6,3,7] so that
        # head h sits at (block h%4, partition offset 64*(h//4)) -- the
        # partition offset then always equals its kv head's offset in kt_sb,
        # satisfying matmul's equal-base-partition requirement.
        # Heads run in pairs with interleaved S/exp/PV chains; Wo matmuls of
        # the previous T_q block are drip-fed into the PE stream (2 per inner
        # iteration) to fill the gaps left by the ACT-bound exp pipeline.

        def wo_steps(tb):
            """Yield fine-grained phase-C steps for T_q block tb."""
            outt_tb = outt[tb % 2]
            for mi in range(TQB // P):
                mt = tb * (TQB // P) + mi
                y_sb = row_p.tile([P, D], F32, tag="rows")
                for nb in range(NB):
                    yp = proj_ps.tile([P, 512], F32, tag="proj")
                    for ks in range(DQ // P):
                        yield ("mm", yp, outt_tb, ks, mi, nb)
                    yield ("evict", yp, y_sb, nb)
                yield ("dma", y_sb, mt)

        def run_wo_step(step):
            kind = step[0]
            if kind == "mm":
                _, yp, outt_tb, ks, mi, nb = step
                nc.tensor.matmul(
                    yp[:], outt_tb[:, ks, mi * P:(mi + 1) * P],
                    wo_sb[:, ks, nb * 512:(nb + 1) * 512],
                    start=(ks == 0), stop=(ks == DQ // P - 1))
            elif kind == "evict":
                _, yp, y_sb, nb = step
                nc.vector.tensor_copy(y_sb[:, nb * 512:(nb + 1) * 512], yp[:])
            else:
                _, y_sb, mt = step
                nc.sync.dma_start(y_v[:, mt, :], y_sb[:])

        outt = [None, None]
        pending = []          # phase-C steps of the previous tb
        for tb in range(NTQB):
            outt_tb = ot_p.tile([P, DQ // P, TQB], BF16, tag="ot")
            outt[tb % 2] = outt_tb
            for hp in range(NHQ // 2):
                heads = (2 * hp, 2 * hp + 1)
                pvs = [
                    pv_ps.tile([DH + 1, TQB], F32, tag="pv", name=f"pv{i}")
                    for i in range(2)]
                exs = [[None, None] for _ in range(NKI // 2)]

                def emit_pv_chain(i, gp):
                    j = heads[i] // GROUP
                    for half in range(2):
                        ki = 2 * gp + half
                        nc.tensor.matmul(
                            pvs[i][:], vones[:, j, ki, :],
                            exs[gp][i][:, half, :],
                            start=(gp == 0 and half == 0),
                            stop=(gp == NKI // 2 - 1 and half == 1))

                for g in range(NKI // 2):
                    # lagged PV and the Wo drip go FIRST each iteration so
                    # the S matmuls (whose score tiles carry a WAR on the
                    # previous group's exp read) issue as late as possible,
                    # giving the Scalar queue maximal drain time.  The first
                    # PV is held to g=2 so it also clears the previous
                    # pair's normalization WAR on the pv banks.
                    if g == 2:
                        emit_pv_chain(0, 0)
                        emit_pv_chain(0, 1)
                        emit_pv_chain(1, 0)
                        emit_pv_chain(1, 1)
                    elif g > 2:
                        emit_pv_chain(0, g - 1)
                        emit_pv_chain(1, g - 1)
                    # drip-feed two Wo matmul steps of the previous block
                    fed = 0
                    while pending and fed < 2:
                        step = pending.pop(0)
                        run_wo_step(step)
                        if step[0] == "mm":
                            fed += 1
                    # S + exp for both chains (group g)
                    for i, h in enumerate(heads):
                        j = h // GROUP
                        mbq, poq = h % 4, (h // GROUP) * DH
                        sp = s_ps.tile([P, 2, TQB], F32, tag="s")
                        for half in range(2):
                            ki = 2 * g + half
                            nc.tensor.matmul(
                                sp[:, half, :],
                                kt_sb[j * DH:(j + 1) * DH, ki * P:(ki + 1) * P],
                                qt_sb[poq:poq + DH, mbq, tb * TQB:(tb + 1) * TQB],
                                start=True, stop=True)
                        ex = exp_p.tile([P, 2, TQB], BF16, tag="exp")
                        nc.scalar.activation(ex[:], sp[:], AF.Exp, scale=SCALE)
                        exs[g][i] = ex
                emit_pv_chain(0, NKI // 2 - 1)
                emit_pv_chain(1, NKI // 2 - 1)
                # normalization for the pair (den staged via SBUF: custom-DVE
                # ops read garbage from PSUM on HW)
                for i, h in enumerate(heads):
                    mbq, poq = h % 4, (h // GROUP) * DH
                    den = rc_p.tile([1, TQB], F32, tag="den")
                    nc.vector.tensor_copy(den[:], pvs[i][DH:DH + 1, :])
                    rc = rc_p.tile([1, TQB], F32, tag="rc")
                    nc.vector.reciprocal_approx_fast(rc[:], den[:])
                    bc = bc_p.tile([DH, TQB], F32, tag="bc")
                    nc.gpsimd.partition_broadcast(bc[:], rc[:], channels=DH)
                    nc.vector.tensor_mul(
                        outt_tb[poq:poq + DH, mbq, :],
                        pvs[i][0:DH, :], bc[:])
            # flush any remaining phase-C work of the previous block, then
            # queue this block's
            for step in pending:
                run_wo_step(step)
            pending = list(wo_steps(tb))
        for step in pending:
            run_wo_step(step)

    nc.compile()
    return nc


_NC_CACHE = {}


def _get_nc():
    if "nc" not in _NC_CACHE:
        _NC_CACHE["nc"] = _build()
    return _NC_CACHE["nc"]


def make_in_maps(x, Wq, Wk, Wv, Wo):
    x = np.ascontiguousarray(np.asarray(x, dtype=np.float32))
    Wq = np.asarray(Wq, dtype=np.float32)
    Wk = np.asarray(Wk, dtype=np.float32)
    Wv = np.asarray(Wv, dtype=np.float32)
    Wo = np.asarray(Wo, dtype=np.float32)

    # interleave the per-core q heads as [0,4,1,5,2,6,3,7] (see phase B note)
    perm = np.concatenate(
        [np.r_[b * DH:(b + 1) * DH, (b + 4) * DH:(b + 5) * DH] for b in range(4)])
    in_maps = []
    for c in range(NCORES):
        b, g = divmod(c, TP)
        in_maps.append({
            "x": x[b],
            "wq": np.ascontiguousarray(
                Wq[:, g * DQ:(g + 1) * DQ][:, perm]).astype(ml_dtypes.bfloat16),
            "wk": np.ascontiguousarray(
                Wk[:, g * DKV:(g + 1) * DKV]).astype(ml_dtypes.bfloat16),
            "wv": np.ascontiguousarray(
                Wv[:, g * DKV:(g + 1) * DKV]).astype(ml_dtypes.bfloat16),
            "wo": np.ascontiguousarray(
                Wo[g * DQ:(g + 1) * DQ, :][perm, :]).astype(ml_dtypes.bfloat16),
        })
    return in_maps


def kernel(x, Wq, Wk, Wv, Wo):
    nc = _get_nc()
    in_maps = make_in_maps(x, Wq, Wk, Wv, Wo)
    res = run_bass_kernel_spmd(nc, in_maps, list(range(NCORES)))
    y = np.zeros((B, T, D), dtype=np.float32)
    for c in range(NCORES):
        b = c // TP
        y[b] += res.results[c]["y"]
    return y


# revision 32
# speedup vs baseline: 1.1808x; 1.1808x over previous
"""GQA attention block on 8 NeuronCores (Trainium2, Bass/Tile).

Sharding: tensor-parallel over head groups (4 ways: 8 q heads / 2 kv heads
per core) x data-parallel over batch (2 ways).  Each core computes a partial
y = attn_out_slice @ Wo_slice for its (batch, head-group); the host sums the
4 TP partials per batch element.

Per-core device program:
  A) x^T via f32r PE transposes (batched 8-per-PSUM-tile, evictions split
     between the Vector and Scalar engines, cast to bf16); q^T/k^T/v^T
     projections in bf16 (fast weight load).  All 4 x row-tiles of a chunk
     are DMA-prefetched up front; weight DMAs are ordered K,V,Q to match
     first use.
  B) attention with two-head software pipelining: heads run in pairs with
     interleaved S -> exp -> PV chains, and PV lags one ki-group behind
     S/exp so the PE never stalls on the Scalar engine.  exp runs on
     [128, 1024] PSUM tiles (2 banks, 2 key-tiles per instruction, scale
     1/sqrt(dh) folded into the activation) to amortize the ~352-cycle ACT
     instruction overhead.  Softmax denominators come from a ones-row in
     the PV weights; normalization = SBUF-staged reciprocal_approx_fast +
     gpsimd partition broadcast + DVE multiply (custom-DVE ops cannot read
     PSUM on hardware).
  C) y = out^T.T @ Wo (bf16), with block tb's 64 matmuls drip-fed 2 per
     inner iteration into block tb+1's attention loop to fill PE gaps and
     keep the PE clock warm.

Measured on trn2: ~497 us HW exec (baseline: 907 us), rel err ~5e-3.
"""

import os
import sys

import ml_dtypes
import numpy as np

for _p in ("/opt/trn_rl_repo",):
    if os.path.isdir(_p) and _p not in sys.path:
        sys.path.insert(0, _p)

from contextlib import ExitStack

import concourse.bass as bass  # noqa: F401  (AP types pulled in transitively)
import concourse.mybir as mybir
import concourse.tile as tile
from concourse import bacc
from concourse.bass_utils import run_bass_kernel_spmd
from concourse.masks import make_identity

P = 128
B, T, D = 2, 2048, 2048
HQ, HKV, DH = 32, 8, 64
GROUP = HQ // HKV            # 4
TP = 4                       # tensor-parallel ways
DP = 2                       # data-parallel ways
NCORES = TP * DP
DQ = D // TP                 # 512 q dims per core (8 heads)
DKV = HKV * DH // TP         # 128 kv dims per core (2 kv heads)
NHQ = HQ // TP               # 8 q heads per core
NKV = HKV // TP              # 2 kv heads per core
NKS = D // P                 # 16 contraction subtiles over D
CH = 512                     # T chunk width in projection phase
NCH = T // CH                # 4
TQB = 512                    # T_q block width in attention / psum bank
NTQB = T // TQB              # 4
NKI = T // P                 # 16 key tiles
NB = D // 512                # 4 output column banks
SCALE = 1.0 / 8.0            # 1/sqrt(DH), folded into Wq on the host
F32 = mybir.dt.float32
F32R = mybir.dt.float32r
BF16 = mybir.dt.bfloat16
FP8 = mybir.dt.float8e4
I32 = mybir.dt.int32
AF = mybir.ActivationFunctionType
PM = mybir.MatmulPerfMode
# Schraudolph fast-exp: bitcast(int32(EXA*x + EXB)) ~ exp(x) (max rel err ~3%)
EXA = 12102203.16 * SCALE     # 2^23/ln2, with the 1/sqrt(dh) folded in
EXB = 127.0 * 2**23 - 366390.0


def _build():
    nc = bacc.Bacc(None, target_bir_lowering=False, debug=False)

    x_ext = nc.dram_tensor("x", [T, D], F32, kind="ExternalInput")
    wq_ext = nc.dram_tensor("wq", [D, DQ], BF16, kind="ExternalInput")
    wk_ext = nc.dram_tensor("wk", [D, DKV], BF16, kind="ExternalInput")
    wv_ext = nc.dram_tensor("wv", [D, DKV], BF16, kind="ExternalInput")
    wo_ext = nc.dram_tensor("wo", [DQ, D], BF16, kind="ExternalInput")
    y_ext = nc.dram_tensor("y", [T, D], F32, kind="ExternalOutput")

    x_v = x_ext[:].rearrange("(to p) d -> p to d", p=P)      # [128,16,2048]
    wq_v = wq_ext[:].rearrange("(ko p) m -> p ko m", p=P)    # [128,16,512]
    wk_v = wk_ext[:].rearrange("(ko p) m -> p ko m", p=P)    # [128,16,128]
    wv_v = wv_ext[:].rearrange("(ko p) m -> p ko m", p=P)
    wo_v = wo_ext[:].rearrange("(ko p) n -> p ko n", p=P)    # [128,4,2048]
    y_v = y_ext[:].rearrange("(to p) n -> p to n", p=P)      # [128,16,2048]

    with tile.TileContext(nc) as tc, ExitStack() as ctx:
        const = ctx.enter_context(tc.tile_pool(name="const", bufs=1))
        big = ctx.enter_context(tc.tile_pool(name="big", bufs=3))
        wkv_p = ctx.enter_context(tc.tile_pool(name="wkv", bufs=1))
        row_p = ctx.enter_context(tc.tile_pool(name="rows", bufs=4))
        qt_p = ctx.enter_context(tc.tile_pool(name="qt", bufs=1))
        kt_p = ctx.enter_context(tc.tile_pool(name="kt", bufs=1))
        vo_p = ctx.enter_context(tc.tile_pool(name="vo", bufs=1))
        exp_p = ctx.enter_context(tc.tile_pool(name="expp", bufs=8))
        bc_p = ctx.enter_context(tc.tile_pool(name="bcp", bufs=2))
        rc_p = ctx.enter_context(tc.tile_pool(name="rcp", bufs=2))
        ot_p = ctx.enter_context(tc.tile_pool(name="otp", bufs=2))

        # PSUM: 8 banks total.
        # s_ps: 2 x [128,1024] (2 banks each) -> 4 banks; attention scores,
        #       also phase-A transpose staging.
        # pv_ps: 2 x [128,512] -> 2 banks; PV accumulators, also phase-A
        #       v-transpose staging.
        # proj_ps: 2 x [128,512] -> 2 banks; projections + Wo outputs.
        s_ps = ctx.enter_context(tc.tile_pool(name="s_ps", bufs=2, space="PSUM"))
        pv_ps = ctx.enter_context(tc.tile_pool(name="pv_ps", bufs=2, space="PSUM"))
        proj_ps = ctx.enter_context(tc.tile_pool(name="proj_ps", bufs=2, space="PSUM"))

        identity = const.tile([P, P], F32)
        make_identity(nc, identity)
        identity_r = const.tile([P, P], F32R)
        nc.vector.tensor_copy(identity_r[:], identity[:])
        id_r = identity_r[:]
        ones_col = const.tile([P, NKV, NKI], F32)
        nc.gpsimd.memset(ones_col[:], 1.0)

        wq_sb = big.tile([P, NKS, DQ], BF16, tag="big")
        wk_sb = wkv_p.tile([P, NKS, DKV], BF16, tag="wk")
        wv_sb = wkv_p.tile([P, NKS, DKV], BF16, tag="wv")

        qt_sb = qt_p.tile([P, DQ // P, T], BF16)   # q^T, [dim, t]
        kt_sb = kt_p.tile([P, T], BF16)            # k^T, [dim(2 kv), t]
        vones = vo_p.tile([P, NKV, NKI, DH + 1], BF16)  # [t%128, kv, t//128, dh|1]
        nc.vector.tensor_copy(vones[:, :, :, DH], ones_col[:])

        # preload the exp table set early so the first phase-B exp is cheap
        warm = rc_p.tile([1, 8], F32, tag="rc")
        nc.gpsimd.memset(warm[:], 0.0)
        nc.scalar.activation(warm[:], warm[:], AF.Exp)

        # ---- Phase A: x^T chunks + projections ----
        for c in range(NCH):
            xt_ch = big.tile([P, NKS, CH], BF16, tag="big")  # x^T[:, c*CH:+CH]
            # prefetch all 4 x row-tiles of this chunk up front
            xrows = []
            for r in range(CH // P):
                xrow = row_p.tile([P, D], F32R, tag="rows", name=f"xrow{r}")
                nc.sync.dma_start(
                    xrow[:], x_v[:, c * (CH // P) + r, :].bitcast(F32R))
                xrows.append(xrow)
            if c == 0:
                # weight DMAs after the first x rows, in need order (K,V,Q);
                # wq in halves so Q-proj mb0/1 can start sooner
                nc.sync.dma_start(wk_sb[:], wk_v)
                nc.sync.dma_start(wv_sb[:], wv_v)
                nc.sync.dma_start(wq_sb[:, :, 0:DQ // 2], wq_v[:, :, 0:DQ // 2])
                nc.sync.dma_start(wq_sb[:, :, DQ // 2:], wq_v[:, :, DQ // 2:])
            for r in range(CH // P):
                xrow_r = xrows[r][:]
                for half in range(2):
                    tp = s_ps.tile([P, 8, P], F32R, tag="s")
                    for k in range(8):
                        dsb = half * 8 + k
                        nc.tensor.transpose(
                            tp[:, k, :], xrow_r[:, dsb * P:(dsb + 1) * P], id_r)
                    # batched eviction of 8 transposes; alternate DVE/ACT
                    dst = xt_ch[:, half * 8:(half + 1) * 8, r * P:(r + 1) * P]
                    if half == 0:
                        nc.vector.tensor_copy(dst, tp[:].bitcast(F32))
                    else:
                        nc.scalar.activation(dst, tp[:].bitcast(F32), AF.Copy)
            # k^T chunk
            kp = proj_ps.tile([P, CH], F32, tag="proj")
            for ks in range(NKS):
                nc.tensor.matmul(kp[:], wk_sb[:, ks, :],
                                 xt_ch[:, ks, :],
                                 start=(ks == 0), stop=(ks == NKS - 1))
            nc.scalar.activation(kt_sb[:, c * CH:(c + 1) * CH], kp[:], AF.Copy)
            # v^T chunk, then PE-transpose into vones (v in natural [t, dh] layout)
            vp = proj_ps.tile([P, CH], F32, tag="proj")
            for ks in range(NKS):
                nc.tensor.matmul(vp[:], wv_sb[:, ks, :],
                                 xt_ch[:, ks, :],
                                 start=(ks == 0), stop=(ks == NKS - 1))
            vt_sb = row_p.tile([P, CH], F32, tag="vt")
            nc.vector.tensor_copy(vt_sb[:], vp[:])
            tpv = pv_ps.tile([P, 4, P], F32, tag="pv")
            for r in range(CH // P):
                nc.tensor.transpose(
                    tpv[:, r, :], vt_sb[:, r * P:(r + 1) * P], identity)
            for j in range(NKV):
                nc.vector.tensor_copy(
                    vones[:, j, c * 4:(c + 1) * 4, 0:DH],
                    tpv[:, :, j * DH:(j + 1) * DH])
            # q^T chunk (scale already folded into Wq host-side)
            for mb in range(DQ // P):
                qp = proj_ps.tile([P, CH], F32, tag="proj")
                for ks in range(NKS):
                    nc.tensor.matmul(
                        qp[:], wq_sb[:, ks, mb * P:(mb + 1) * P],
                        xt_ch[:, ks, :],
                        start=(ks == 0), stop=(ks == NKS - 1))
                nc.vector.tensor_copy(qt_sb[:, mb, c * CH:(c + 1) * CH], qp[:])

        # prefetch Wo (slot freed by wq after phase A)
        wo_sb = big.tile([P, DQ // P, D], BF16, tag="big")
        nc.sync.dma_start(wo_sb[:], wo_v)

        # ---- Phases B+C interleaved per T_q block ----
        # q heads are permuted host-side to order [0,4,1,5,2,6,3,7] so that
        # head h sits at (block h%4, partition offset 64*(h//4)) -- the
        # partition offset then always equals its kv head's offset in kt_sb,
        # satisfying matmul's equal-base-partition requirement.
        # Heads run in pairs with interleaved S/exp/PV chains; Wo matmuls of
        # the previous T_q block are drip-fed into the PE stream (2 per inner
        # iteration) to fill the gaps left by the ACT-bound exp pipeline.

        def wo_steps(tb):
            """Yield fine-grained phase-C steps for T_q block tb."""
            outt_tb = outt[tb % 2]
            for mi in range(TQB // P):
                mt = tb * (TQB // P) + mi
                y_sb = row_p.tile([P, D], F32, tag="rows")
                for nb in range(NB):
                    yp = proj_ps.tile([P, 512], F32, tag="proj")
                    for ks in range(DQ // P):
                        yield ("mm", yp, outt_tb, ks, mi, nb)
                    yield ("evict", yp, y_sb, nb)
                    if nb == 1:
                        yield ("dma2", y_sb, mt, 0)
                yield ("dma2", y_sb, mt, 1)

        def run_wo_step(step):
            kind = step[0]
            if kind == "mm":
                _, yp, outt_tb, ks, mi, nb = step
                nc.tensor.matmul(
                    yp[:], outt_tb[:, ks, mi * P:(mi + 1) * P],
                    wo_sb[:, ks, nb * 512:(nb + 1) * 512],
                    start=(ks == 0), stop=(ks == DQ // P - 1))
            elif kind == "evict":
                _, yp, y_sb, nb = step
                nc.vector.tensor_copy(y_sb[:, nb * 512:(nb + 1) * 512], yp[:])
            else:
                _, y_sb, mt, h = step
                nc.sync.dma_start(y_v[:, mt, h * 1024:(h + 1) * 1024],
                                  y_sb[:, h * 1024:(h + 1) * 1024])

        outt = [None, None]
        pending = []          # phase-C steps of the previous tb
        for tb in range(NTQB):
            outt_tb = ot_p.tile([P, DQ // P, TQB], BF16, tag="ot")
            outt[tb % 2] = outt_tb
            for hp in range(NHQ // 2):
                heads = (2 * hp, 2 * hp + 1)
                pvs = [
                    pv_ps.tile([DH + 1, TQB], F32, tag="pv", name=f"pv{i}")
                    for i in range(2)]
                exs = [[None, None] for _ in range(NKI // 2)]

                def emit_pv_chain(i, gp):
                    j = heads[i] // GROUP
                    for half in range(2):
                        ki = 2 * gp + half
                        nc.tensor.matmul(
                            pvs[i][:], vones[:, j, ki, :],
                            exs[gp][i][:, half, :],
                            start=(gp == 0 and half == 0),
                            stop=(gp == NKI // 2 - 1 and half == 1))

                for g in range(NKI // 2):
                    # S + exp for both chains (group g)
                    for i, h in enumerate(heads):
                        j = h // GROUP
                        mbq, poq = h % 4, (h // GROUP) * DH
                        sp = s_ps.tile([P, 2, TQB], F32, tag="s")
                        for half in range(2):
                            ki = 2 * g + half
                            nc.tensor.matmul(
                                sp[:, half, :],
                                kt_sb[j * DH:(j + 1) * DH, ki * P:(ki + 1) * P],
                                qt_sb[poq:poq + DH, mbq, tb * TQB:(tb + 1) * TQB],
                                start=True, stop=True)
                        ex = exp_p.tile([P, 2, TQB], BF16, tag="exp")
                        nc.scalar.activation(ex[:], sp[:], AF.Exp, scale=SCALE)
                        exs[g][i] = ex
                    # PV runs one group behind S/exp so the PE never
                    # stalls on the Scalar engine; the first PV is further
                    # delayed to g=2 so it never waits on the previous
                    # pair's normalization to release the pv banks
                    if g == 2:
                        emit_pv_chain(0, 0)
                        emit_pv_chain(0, 1)
                        emit_pv_chain(1, 0)
                        emit_pv_chain(1, 1)
                    elif g > 2:
                        emit_pv_chain(0, g - 1)
                        emit_pv_chain(1, g - 1)
                    # drip-feed two Wo matmul steps of the previous block
                    fed = 0
                    while pending and fed < 2:
                        step = pending.pop(0)
                        run_wo_step(step)
                        if step[0] == "mm":
                            fed += 1
                # final PV + normalization per chain: chain 0's norm (on
                # DVE/gpsimd) overlaps chain 1's last PV on the PE.  den is
                # staged via SBUF (custom-DVE ops read garbage from PSUM).
                for i, h in enumerate(heads):
                    emit_pv_chain(i, NKI // 2 - 1)
                    mbq, poq = h % 4, (h // GROUP) * DH
                    den = rc_p.tile([1, TQB], F32, tag="den")
                    nc.vector.tensor_copy(den[:], pvs[i][DH:DH + 1, :])
                    rc = rc_p.tile([1, TQB], F32, tag="rc")
                    nc.vector.reciprocal_approx_fast(rc[:], den[:])
                    bc = bc_p.tile([DH, TQB], F32, tag="bc")
                    nc.gpsimd.partition_broadcast(bc[:], rc[:], channels=DH)
                    nc.vector.tensor_mul(
                        outt_tb[poq:poq + DH, mbq, :],
                        pvs[i][0:DH, :], bc[:])
            # flush any remaining phase-C work of the previous block, then
            # queue this block's
            for step in pending:
                run_wo_step(step)
            pending = list(wo_steps(tb))
        for step in pending:
            run_wo_step(step)

    nc.compile()
    return nc


_NC_CACHE = {}


def _get_nc():
    if "nc" not in _NC_CACHE:
        _NC_CACHE["nc"] = _build()
    return _NC_CACHE["nc"]


def make_in_maps(x, Wq, Wk, Wv, Wo):
    x = np.ascontiguousarray(np.asarray(x, dtype=np.float32))
    Wq = np.asarray(Wq, dtype=np.float32)
    Wk = np.asarray(Wk, dtype=np.float32)
    Wv = np.asarray(Wv, dtype=np.float32)
    Wo = np.asarray(Wo, dtype=np.float32)

    # interleave the per-core q heads as [0,4,1,5,2,6,3,7] (see phase B note)
    perm = np.concatenate(
        [np.r_[b * DH:(b + 1) * DH, (b + 4) * DH:(b + 5) * DH] for b in range(4)])
    in_maps = []
    for c in range(NCORES):
        b, g = divmod(c, TP)
        in_maps.append({
            "x": x[b],
            "wq": np.ascontiguousarray(
                Wq[:, g * DQ:(g + 1) * DQ][:, perm]).astype(ml_dtypes.bfloat16),
            "wk": np.ascontiguousarray(
                Wk[:, g * DKV:(g + 1) * DKV]).astype(ml_dtypes.bfloat16),
            "wv": np.ascontiguousarray(
                Wv[:, g * DKV:(g + 1) * DKV]).astype(ml_dtypes.bfloat16),
            "wo": np.ascontiguousarray(
                Wo[g * DQ:(g + 1) * DQ, :][perm, :]).astype(ml_dtypes.bfloat16),
        })
    return in_maps


def kernel(x, Wq, Wk, Wv, Wo):
    nc = _get_nc()
    in_maps = make_in_maps(x, Wq, Wk, Wv, Wo)
    res = run_bass_kernel_spmd(nc, in_maps, list(range(NCORES)))
    y = np.zeros((B, T, D), dtype=np.float32)
    for c in range(NCORES):
        b = c // TP
        y[b] += res.results[c]["y"]
    return y


# revision 33
# speedup vs baseline: 1.2191x; 1.0325x over previous
"""GQA attention block on 8 NeuronCores (Trainium2, Bass/Tile).

Sharding: tensor-parallel over head groups (4 ways: 8 q heads / 2 kv heads
per core) x data-parallel over batch (2 ways).  Each core computes a partial
y = attn_out_slice @ Wo_slice for its (batch, head-group); the host sums the
4 TP partials per batch element.

Per-core device program:
  A) x^T via f32r PE transposes (batched 8-per-PSUM-tile, evictions split
     between the Vector and Scalar engines, cast to bf16); q^T/k^T/v^T
     projections in bf16 (fast weight load).  All 4 x row-tiles of a chunk
     are DMA-prefetched up front; weight DMAs are ordered K,V,Q to match
     first use.
  B) attention with two-head software pipelining: heads run in pairs with
     interleaved S -> exp -> PV chains, and PV lags one ki-group behind
     S/exp so the PE never stalls on the Scalar engine.  exp runs on
     [128, 1024] PSUM tiles (2 banks, 2 key-tiles per instruction, scale
     1/sqrt(dh) folded into the activation) to amortize the ~352-cycle ACT
     instruction overhead.  Softmax denominators come from a ones-row in
     the PV weights; normalization = SBUF-staged reciprocal_approx_fast +
     gpsimd partition broadcast + DVE multiply (custom-DVE ops cannot read
     PSUM on hardware).
  C) y = out^T.T @ Wo (bf16), with block tb's 64 matmuls drip-fed 2 per
     inner iteration into block tb+1's attention loop to fill PE gaps and
     keep the PE clock warm.

Measured on trn2: ~497 us HW exec (baseline: 907 us), rel err ~5e-3.
"""

import os
import sys

import ml_dtypes
import numpy as np

for _p in ("/opt/trn_rl_repo",):
    if os.path.isdir(_p) and _p not in sys.path:
        sys.path.insert(0, _p)

from contextlib import ExitStack

import concourse.bass as bass  # noqa: F401  (AP types pulled in transitively)
import concourse.mybir as mybir
import concourse.tile as tile
from concourse import bacc
from concourse.bass_utils import run_bass_kernel_spmd
from concourse.masks import make_identity

P = 128
B, T, D = 2, 2048, 2048
HQ, HKV, DH = 32, 8, 64
GROUP = HQ // HKV            # 4
TP = 4                       # tensor-parallel ways
DP = 2                       # data-parallel ways
NCORES = TP * DP
DQ = D // TP                 # 512 q dims per core (8 heads)
DKV = HKV * DH // TP         # 128 kv dims per core (2 kv heads)
NHQ = HQ // TP               # 8 q heads per core
NKV = HKV // TP              # 2 kv heads per core
NKS = D // P                 # 16 contraction subtiles over D
CH = 512                     # T chunk width in projection phase
NCH = T // CH                # 4
TQB = 512                    # T_q block width in attention / psum bank
NTQB = T // TQB              # 4
NKI = T // P                 # 16 key tiles
NB = D // 512                # 4 output column banks
SCALE = 1.0 / 8.0            # 1/sqrt(DH), folded into Wq on the host
F32 = mybir.dt.float32
F32R = mybir.dt.float32r
BF16 = mybir.dt.bfloat16
FP8 = mybir.dt.float8e4
I32 = mybir.dt.int32
AF = mybir.ActivationFunctionType
PM = mybir.MatmulPerfMode
# Schraudolph fast-exp: bitcast(int32(EXA*x + EXB)) ~ exp(x) (max rel err ~3%)
EXA = 12102203.16 * SCALE     # 2^23/ln2, with the 1/sqrt(dh) folded in
EXB = 127.0 * 2**23 - 366390.0


def _build():
    nc = bacc.Bacc(None, target_bir_lowering=False, debug=False)

    x_ext = nc.dram_tensor("x", [T, D], F32, kind="ExternalInput")
    wq_ext = nc.dram_tensor("wq", [D, DQ], BF16, kind="ExternalInput")
    wk_ext = nc.dram_tensor("wk", [D, DKV], BF16, kind="ExternalInput")
    wv_ext = nc.dram_tensor("wv", [D, DKV], BF16, kind="ExternalInput")
    wo_ext = nc.dram_tensor("wo", [DQ, D], BF16, kind="ExternalInput")
    y_ext = nc.dram_tensor("y", [T, D], F32, kind="ExternalOutput")

    x_v = x_ext[:].rearrange("(to p) d -> p to d", p=P)      # [128,16,2048]
    wq_v = wq_ext[:].rearrange("(ko p) m -> p ko m", p=P)    # [128,16,512]
    wk_v = wk_ext[:].rearrange("(ko p) m -> p ko m", p=P)    # [128,16,128]
    wv_v = wv_ext[:].rearrange("(ko p) m -> p ko m", p=P)
    wo_v = wo_ext[:].rearrange("(ko p) n -> p ko n", p=P)    # [128,4,2048]
    y_v = y_ext[:].rearrange("(to p) n -> p to n", p=P)      # [128,16,2048]

    with tile.TileContext(nc) as tc, ExitStack() as ctx:
        const = ctx.enter_context(tc.tile_pool(name="const", bufs=1))
        big = ctx.enter_context(tc.tile_pool(name="big", bufs=3))
        wkv_p = ctx.enter_context(tc.tile_pool(name="wkv", bufs=1))
        row_p = ctx.enter_context(tc.tile_pool(name="rows", bufs=4))
        qt_p = ctx.enter_context(tc.tile_pool(name="qt", bufs=1))
        kt_p = ctx.enter_context(tc.tile_pool(name="kt", bufs=1))
        vo_p = ctx.enter_context(tc.tile_pool(name="vo", bufs=1))
        exp_p = ctx.enter_context(tc.tile_pool(name="expp", bufs=8))
        bc_p = ctx.enter_context(tc.tile_pool(name="bcp", bufs=2))
        rc_p = ctx.enter_context(tc.tile_pool(name="rcp", bufs=2))
        ot_p = ctx.enter_context(tc.tile_pool(name="otp", bufs=2))

        # PSUM: 8 banks total.
        # s_ps: 2 x [128,1024] (2 banks each) -> 4 banks; attention scores,
        #       also phase-A transpose staging.
        # pv_ps: 2 x [128,512] -> 2 banks; PV accumulators, also phase-A
        #       v-transpose staging.
        # proj_ps: 2 x [128,512] -> 2 banks; projections + Wo outputs.
        s_ps = ctx.enter_context(tc.tile_pool(name="s_ps", bufs=2, space="PSUM"))
        pv_ps = ctx.enter_context(tc.tile_pool(name="pv_ps", bufs=2, space="PSUM"))
        proj_ps = ctx.enter_context(tc.tile_pool(name="proj_ps", bufs=2, space="PSUM"))

        identity = const.tile([P, P], F32)
        make_identity(nc, identity)
        identity_r = const.tile([P, P], F32R)
        nc.vector.tensor_copy(identity_r[:], identity[:])
        id_r = identity_r[:]
        ones_col = const.tile([P, NKV, NKI], F32)
        nc.gpsimd.memset(ones_col[:], 1.0)

        wq_sb = big.tile([P, NKS, DQ], BF16, tag="big")
        wk_sb = wkv_p.tile([P, NKS, DKV], BF16, tag="wk")
        wv_sb = wkv_p.tile([P, NKS, DKV], BF16, tag="wv")

        qt_sb = qt_p.tile([P, DQ // P, T], BF16)   # q^T, [dim, t]
        kt_sb = kt_p.tile([P, T], BF16)            # k^T, [dim(2 kv), t]
        vones = vo_p.tile([P, NKV, NKI, DH + 1], BF16)  # [t%128, kv, t//128, dh|1]
        nc.vector.tensor_copy(vones[:, :, :, DH], ones_col[:])

        # preload the exp table set early so the first phase-B exp is cheap
        warm = rc_p.tile([1, 8], F32, tag="rc")
        nc.gpsimd.memset(warm[:], 0.0)
        nc.scalar.activation(warm[:], warm[:], AF.Exp)

        # ---- Phase A: x^T chunks + projections ----
        for c in range(NCH):
            xt_ch = big.tile([P, NKS, CH], BF16, tag="big")  # x^T[:, c*CH:+CH]
            # prefetch all 4 x row-tiles of this chunk up front
            xrows = []
            for r in range(CH // P):
                xrow = row_p.tile([P, D], F32R, tag="rows", name=f"xrow{r}")
                nc.sync.dma_start(
                    xrow[:], x_v[:, c * (CH // P) + r, :].bitcast(F32R))
                xrows.append(xrow)
            if c == 0:
                # weight DMAs after the first x rows, in need order (K,V,Q);
                # wq in halves so Q-proj mb0/1 can start sooner
                nc.sync.dma_start(wk_sb[:], wk_v)
                nc.sync.dma_start(wv_sb[:], wv_v)
                nc.sync.dma_start(wq_sb[:, :, 0:DQ // 2], wq_v[:, :, 0:DQ // 2])
                nc.sync.dma_start(wq_sb[:, :, DQ // 2:], wq_v[:, :, DQ // 2:])
            for r in range(CH // P):
                xrow_r = xrows[r][:]
                for half in range(2):
                    tp = s_ps.tile([P, 8, P], F32R, tag="s")
                    for k in range(8):
                        dsb = half * 8 + k
                        nc.tensor.transpose(
                            tp[:, k, :], xrow_r[:, dsb * P:(dsb + 1) * P], id_r)
                    # batched eviction of 8 transposes; alternate DVE/ACT
                    dst = xt_ch[:, half * 8:(half + 1) * 8, r * P:(r + 1) * P]
                    if half == 0:
                        nc.vector.tensor_copy(dst, tp[:].bitcast(F32))
                    else:
                        nc.scalar.activation(dst, tp[:].bitcast(F32), AF.Copy)
            # k^T chunk
            kp = proj_ps.tile([P, CH], F32, tag="proj")
            for ks in range(NKS):
                nc.tensor.matmul(kp[:], wk_sb[:, ks, :],
                                 xt_ch[:, ks, :],
                                 start=(ks == 0), stop=(ks == NKS - 1))
            nc.scalar.activation(kt_sb[:, c * CH:(c + 1) * CH], kp[:], AF.Copy)
            # v^T chunk, then PE-transpose into vones (v in natural [t, dh] layout)
            vp = proj_ps.tile([P, CH], F32, tag="proj")
            for ks in range(NKS):
                nc.tensor.matmul(vp[:], wv_sb[:, ks, :],
                                 xt_ch[:, ks, :],
                                 start=(ks == 0), stop=(ks == NKS - 1))
            vt_sb = row_p.tile([P, CH], F32, tag="vt")
            nc.vector.tensor_copy(vt_sb[:], vp[:])
            tpv = pv_ps.tile([P, 4, P], F32, tag="pv")
            for r in range(CH // P):
                nc.tensor.transpose(
                    tpv[:, r, :], vt_sb[:, r * P:(r + 1) * P], identity)
            for j in range(NKV):
                nc.vector.tensor_copy(
                    vones[:, j, c * 4:(c + 1) * 4, 0:DH],
                    tpv[:, :, j * DH:(j + 1) * DH])
            # q^T chunk (scale already folded into Wq host-side)
            for mb in range(DQ // P):
                qp = proj_ps.tile([P, CH], F32, tag="proj")
                for ks in range(NKS):
                    nc.tensor.matmul(
                        qp[:], wq_sb[:, ks, mb * P:(mb + 1) * P],
                        xt_ch[:, ks, :],
                        start=(ks == 0), stop=(ks == NKS - 1))
                nc.vector.tensor_copy(qt_sb[:, mb, c * CH:(c + 1) * CH], qp[:])

        # prefetch Wo (slot freed by wq after phase A)
        wo_sb = big.tile([P, DQ // P, D], BF16, tag="big")
        nc.sync.dma_start(wo_sb[:], wo_v)

        # ---- Phases B+C interleaved per T_q block ----
        # q heads are permuted host-side to order [0,4,1,5,2,6,3,7] so that
        # head h sits at (block h%4, partition offset 64*(h//4)) -- the
        # partition offset then always equals its kv head's offset in kt_sb,
        # satisfying matmul's equal-base-partition requirement.
        # Heads run in pairs with interleaved S/exp/PV chains; Wo matmuls of
        # the previous T_q block are drip-fed into the PE stream (2 per inner
        # iteration) to fill the gaps left by the ACT-bound exp pipeline.

        def wo_steps(tb):
            """Yield fine-grained phase-C steps for T_q block tb."""
            outt_tb = outt[tb % 2]
            for mi in range(TQB // P):
                mt = tb * (TQB // P) + mi
                y_sb = row_p.tile([P, D], F32, tag="rows")
                for nb in range(NB):
                    yp = proj_ps.tile([P, 512], F32, tag="proj")
                    for ks in range(DQ // P):
                        yield ("mm", yp, outt_tb, ks, mi, nb)
                    yield ("evict", yp, y_sb, nb)
                yield ("dma", y_sb, mt)

        def run_wo_step(step):
            kind = step[0]
            if kind == "mm":
                _, yp, outt_tb, ks, mi, nb = step
                nc.tensor.matmul(
                    yp[:], outt_tb[:, ks, mi * P:(mi + 1) * P],
                    wo_sb[:, ks, nb * 512:(nb + 1) * 512],
                    start=(ks == 0), stop=(ks == DQ // P - 1))
            elif kind == "evict":
                _, yp, y_sb, nb = step
                nc.vector.tensor_copy(y_sb[:, nb * 512:(nb + 1) * 512], yp[:])
            else:
                _, y_sb, mt = step
                nc.sync.dma_start(y_v[:, mt, :], y_sb[:])

        outt = [None, None]
        pending = []          # phase-C steps of the previous tb
        for tb in range(NTQB):
            outt_tb = ot_p.tile([P, DQ // P, TQB], BF16, tag="ot")
            outt[tb % 2] = outt_tb
            for hp in range(NHQ // 2):
                heads = (2 * hp, 2 * hp + 1)
                pvs = [
                    pv_ps.tile([DH + 1, TQB], F32, tag="pv", name=f"pv{i}")
                    for i in range(2)]
                exs = [[None, None] for _ in range(NKI // 2)]

                def emit_pv_chain(i, gp):
                    j = heads[i] // GROUP
                    for half in range(2):
                        ki = 2 * gp + half
                        nc.tensor.matmul(
                            pvs[i][:], vones[:, j, ki, :],
                            exs[gp][i][:, half, :],
                            start=(gp == 0 and half == 0),
                            stop=(gp == NKI // 2 - 1 and half == 1))

                for g in range(NKI // 2):
                    # S + exp for both chains (group g)
                    for i, h in enumerate(heads):
                        j = h // GROUP
                        mbq, poq = h % 4, (h // GROUP) * DH
                        sp = s_ps.tile([P, 2, TQB], F32, tag="s")
                        for half in range(2):
                            ki = 2 * g + half
                            nc.tensor.matmul(
                                sp[:, half, :],
                                kt_sb[j * DH:(j + 1) * DH, ki * P:(ki + 1) * P],
                                qt_sb[poq:poq + DH, mbq, tb * TQB:(tb + 1) * TQB],
                                start=True, stop=True)
                        ex = exp_p.tile([P, 2, TQB], BF16, tag="exp")
                        nc.scalar.activation(ex[:], sp[:], AF.Exp, scale=SCALE)
                        exs[g][i] = ex
                    # PV runs one group behind S/exp so the PE never
                    # stalls on the Scalar engine; the first PV is further
                    # delayed to g=2 so it never waits on the previous
                    # pair's normalization to release the pv banks
                    if g == 2:
                        emit_pv_chain(0, 0)
                        emit_pv_chain(0, 1)
                        emit_pv_chain(1, 0)
                        emit_pv_chain(1, 1)
                    elif g > 2:
                        emit_pv_chain(0, g - 1)
                        emit_pv_chain(1, g - 1)
                    # drip-feed two Wo matmul steps of the previous block
                    fed = 0
                    while pending and fed < 2:
                        step = pending.pop(0)
                        run_wo_step(step)
                        if step[0] == "mm":
                            fed += 1
                emit_pv_chain(0, NKI // 2 - 1)
                emit_pv_chain(1, NKI // 2 - 1)
                # normalization for the pair (den staged via SBUF: custom-DVE
                # ops read garbage from PSUM on HW)
                for i, h in enumerate(heads):
                    mbq, poq = h % 4, (h // GROUP) * DH
                    den = rc_p.tile([1, TQB], F32, tag="den")
                    nc.vector.tensor_copy(den[:], pvs[i][DH:DH + 1, :])
                    rc = rc_p.tile([1, TQB], F32, tag="rc")
                    nc.vector.reciprocal_approx_fast(rc[:], den[:])
                    bc = bc_p.tile([DH, TQB], F32, tag="bc")
                    nc.gpsimd.partition_broadcast(bc[:], rc[:], channels=DH)
                    nc.vector.tensor_mul(
                        outt_tb[poq:poq + DH, mbq, :],
                        pvs[i][0:DH, :], bc[:])
            # flush any remaining phase-C work of the previous block, then
            # queue this block's
            for step in pending:
                run_wo_step(step)
            pending = list(wo_steps(tb))
        for step in pending:
            run_wo_step(step)

    nc.compile()
    return nc


_NC_CACHE = {}


def _get_nc():
    if "nc" not in _NC_CACHE:
        _NC_CACHE["nc"] = _build()
    return _NC_CACHE["nc"]


def make_in_maps(x, Wq, Wk, Wv, Wo):
    x = np.ascontiguousarray(np.asarray(x, dtype=np.float32))
    Wq = np.asarray(Wq, dtype=np.float32)
    Wk = np.asarray(Wk, dtype=np.float32)
    Wv = np.asarray(Wv, dtype=np.float32)
    Wo = np.asarray(Wo, dtype=np.float32)

    # interleave the per-core q heads as [0,4,1,5,2,6,3,7] (see phase B note)
    perm = np.concatenate(
        [np.r_[b * DH:(b + 1) * DH, (b + 4) * DH:(b + 5) * DH] for b in range(4)])
    in_maps = []
    for c in range(NCORES):
        b, g = divmod(c, TP)
        in_maps.append({
            "x": x[b],
            "wq": np.ascontiguousarray(
                Wq[:, g * DQ:(g + 1) * DQ][:, perm]).astype(ml_dtypes.bfloat16),
            "wk": np.ascontiguousarray(
                Wk[:, g * DKV:(g + 1) * DKV]).astype(ml_dtypes.bfloat16),
            "wv": np.ascontiguousarray(
                Wv[:, g * DKV:(g + 1) * DKV]).astype(ml_dtypes.bfloat16),
            "wo": np.ascontiguousarray(
                Wo[g * DQ:(g + 1) * DQ, :][perm, :]).astype(ml_dtypes.bfloat16),
        })
    return in_maps


def kernel(x, Wq, Wk, Wv, Wo):
    nc = _get_nc()
    in_maps = make_in_maps(x, Wq, Wk, Wv, Wo)
    res = run_bass_kernel_spmd(nc, in_maps, list(range(NCORES)))
    y = np.zeros((B, T, D), dtype=np.float32)
    for c in range(NCORES):
        b = c // TP
        y[b] += res.results[c]["y"]
    return y
